# revision 19
# baseline (speedup 1.0000x reference)
"""Trainium2 Bass kernel for ConditionCrossAttention2D.

Reference computation (per batch item b, with n = H*W spatial positions):
    q = Wq @ cond + bq            # [Ck, n] -> used as q[n, Ck]
    k = Wk @ feat + bk            # [Ck, n]
    v = Wv @ feat + bv            # [C, n]
    energy[i, j] = sum_ck q[ck, i] * k[ck, j]
    attn = softmax_j(energy)
    out[c, i] = sum_j v[c, j] * attn[i, j]
    result = gamma * out + feat

Sharding: 8 cores = (batch b in 0..3) x (query-half h in 0..1). Each core
computes the full [2048 x 4096] attention for its query half - no
cross-core communication.

Per-core design (v2 - pipelined exp):
  - energy is computed TRANSPOSED: e_T[j, i] (keys on partitions); the
    exp'd tile attnT[j, i] is directly the stationary operand of the PV
    matmul out[i, c] = sum_j attnT[j, i] * vT[j, c].
  - softmax denominators come free from an appended ones-column in vT.
  - max-subtraction is skipped (energies are O(1); exp computed in fp32).
  - THE key bottleneck is the scalar engine's exp (1 elem/lane/cycle,
    63us of ACT time for 8.4M exps/core). The energy PSUM is split into
    two independent 2-bank halves A (queries 0..1023, PE row groups
    0/32) and B (queries 1024..2047, row groups 64/96) with one
    ACTIVATE each, so the energy matmuls of unit u+1's A-half overlap
    the exp of unit u's B-half: ACT runs near back-to-back instead of
    serializing exp -> energy-mm -> exp on a single 4-bank buffer.
  - biases are folded into the projection matmul chains as K=1 rank-1
    update matmuls (bias row x ones row), so PSUM evacuation is a cheap
    2x-mode tensor_copy instead of 1x tensor_scalar adds on DVE.
  - input DMAs are spread across the Sync (weights+cond), GpSimd (feat,
    residual) and Scalar (small consts + wv, output stores) queues: DMA
    instruction issue costs ~0.6us each and was serializing the
    prologue on a single queue.
  - a dummy exp on a const preloads the ACT exp table (~2.7us) during
    the DMA prologue.
"""

import os
from contextlib import ExitStack

import numpy as np

import concourse.bass as bass
import concourse.tile as tile
from concourse import mybir
from concourse.bass_utils import run_bass_kernel_spmd
from concourse.masks import make_identity

B, C, CK, H, W = 4, 256, 32, 64, 64
N = H * W            # 4096 spatial positions
NCORES = 8
NL = N // 2          # 2048 queries per core
P = 128
NJT = N // P         # 32 key tiles
NIT = NL // P        # 16 query tiles per core
GJ = 8               # key tiles per group
NG = NJT // GJ       # 4 groups
F32 = mybir.dt.float32
BF16 = mybir.dt.bfloat16
EXP = mybir.ActivationFunctionType.Exp
ADD = mybir.AluOpType.add
MULT = mybir.AluOpType.mult

LAST_EXEC_TIME_NS = None
LAST_TRACE = None

ts = bass.ts


def _emit(tc, ctx):
    nc = tc.nc

    feat_d = nc.declare_dram_parameter("feat", [C, N], BF16, isOutput=False)
    cond_d = nc.declare_dram_parameter("cond", [C, NL], BF16, isOutput=False)
    fres_d = nc.declare_dram_parameter("fres", [C, NL], F32, isOutput=False)
    # Wv/bv arrive pre-scaled by gamma from the host (reparametrization:
    # gamma*(attn@v) + feat == attn@(gamma*v) + feat), so the finalize
    # stage is a plain residual add.
    wq_d = nc.declare_dram_parameter("Wq", [CK, C], F32, isOutput=False)
    wk_d = nc.declare_dram_parameter("Wk", [CK, C], F32, isOutput=False)
    wv_d = nc.declare_dram_parameter("Wv", [C, C], F32, isOutput=False)
    bq_d = nc.declare_dram_parameter("bq", [CK], F32, isOutput=False)
    bk_d = nc.declare_dram_parameter("bk", [CK], F32, isOutput=False)
    bv_d = nc.declare_dram_parameter("bv", [C], F32, isOutput=False)
    out_d = nc.declare_dram_parameter("out", [C, NL], F32, isOutput=True)

    def bcast_ap(handle, parts, free):
        ap = handle[:]
        return bass.AP(tensor=ap.tensor, offset=ap.offset, ap=[[0, parts], [1, free]])

    def row_ap(handle, free):
        ap = handle[:]
        return bass.AP(tensor=ap.tensor, offset=ap.offset, ap=[[free, 1], [1, free]])

    consts = ctx.enter_context(tc.tile_pool(name="consts", bufs=1))
    persist = ctx.enter_context(tc.tile_pool(name="persist", bufs=1))
    loads = ctx.enter_context(tc.tile_pool(name="loads", bufs=1))
    attnp = ctx.enter_context(tc.tile_pool(name="attn", bufs=2))
    finp = ctx.enter_context(tc.tile_pool(name="fin", bufs=3))
    stagep = ctx.enter_context(tc.tile_pool(name="stage", bufs=2))
    # PSUM (8 banks): energy halves A/B 2 banks each, pv 2x1, transpose 2x1.
    epA = ctx.enter_context(tc.tile_pool(name="epA", bufs=1, space="PSUM"))
    epB = ctx.enter_context(tc.tile_pool(name="epB", bufs=1, space="PSUM"))
    pvp = ctx.enter_context(tc.tile_pool(name="pvps", bufs=2, space="PSUM"))
    tpp = ctx.enter_context(tc.tile_pool(name="tpps", bufs=2, space="PSUM"))

    ident = consts.tile([P, P], F32)
    make_identity(nc, ident)

    # Preload the ACT exp table set during the DMA prologue (first real
    # exp would otherwise stall ~2.7us on the table DMA).
    warm = consts.tile([1, 1], F32)
    nc.scalar.activation(warm[:], ident[0:1, 0:1], EXP)

    # Scratch operands for the PE warm-up / filler matmuls (the HAM clock
    # gate keeps the PE at 1.2 GHz until it sees ~3.4us of near-continuous
    # matmul activity; idle-ish windows re-throttle it to half clock).
    # memset on gpsimd: the vector queue starts ~1.5us later.
    wsrc = consts.tile([P, 512], BF16)
    nc.gpsimd.memset(wsrc[:], 0.0)

    def pe_filler(n):
        for _ in range(n):
            wps = pvp.tile([P, 512], F32, tag="pv", name="wps")
            nc.tensor.matmul(wps[:], wsrc[:, 0:P], wsrc[:],
                             start=True, stop=True)

    # Transposed weights (bf16): wq_t[p, ct, 32r+ck] = Wq[ck, ct*128+p]
    # for replica r in {0..3} (feeds the packed energy matmuls).
    wq_t = consts.tile([P, 2, 4 * CK], BF16)
    wk_t = consts.tile([P, 2, 4 * CK], BF16)
    # wv_t[p, ct, c] = Wv[c, ct*128+p]; column 256 stays 0
    wv_t = consts.tile([P, 2, C + 1], BF16)
    nc.vector.memset(wv_t[:], 0.0)

    # bv broadcast across partitions; column 256 = 1.0 (ones column of vT)
    bv_b = consts.tile([P, C + 1], F32)
    nc.vector.memset(bv_b[:], 1.0)
    # per-partition bias columns, bq/bk replicated x4 down the partitions
    bq_c = consts.tile([4 * CK, 1], F32)
    bk_c = consts.tile([4 * CK, 1], F32)

    # ---- input loads, split across engine DMA queues ----
    # sync queue: Wq, Wk, then cond chunks (the q-projection critical path)
    wq_raw = loads.tile([CK, C], F32)
    nc.sync.dma_start(out=wq_raw[:], in_=wq_d[:, :])
    wk_raw = loads.tile([CK, C], F32)
    nc.sync.dma_start(out=wk_raw[:], in_=wk_d[:, :])

    # one 3D DMA per 512-col chunk (DMA instruction issue costs ~0.6us
    # each on the engine queue, so fewer+bigger wins the prologue)
    def load_chunk(eng, dram, ncols, col0, tag):
        t = loads.tile([P, 2, 512], BF16, tag=tag, name="in_bf")
        ap = dram[:, :]
        src = bass.AP(tensor=ap.tensor, offset=col0,
                      ap=[[ncols, P], [P * ncols, 2], [1, 512]])
        eng.dma_start(out=t[:], in_=src)
        return t

    cond_c = [load_chunk(nc.sync if icc < 2 else nc.scalar,
                         cond_d, NL, icc * 512, f"cond{icc}")
              for icc in range(NL // 512)]

    # scalar queue: small consts + Wv (idle until the first exp).
    # bq/bk are loaded 4x-replicated down the partitions with a single
    # repeating-source DMA each.
    def rep4_ap(handle):
        ap = handle[:]
        return bass.AP(tensor=ap.tensor, offset=ap.offset,
                       ap=[[0, 4], [1, CK]])

    nc.scalar.dma_start(out=bq_c[:], in_=rep4_ap(bq_d))
    nc.scalar.dma_start(out=bk_c[:], in_=rep4_ap(bk_d))
    nc.scalar.dma_start(out=bv_b[:, 0:C], in_=bcast_ap(bv_d, P, C))
    wv_raw = loads.tile([P, 2, C], F32)
    for cb in range(2):
        nc.scalar.dma_start(out=wv_raw[:, cb, :], in_=wv_d[ts(cb, P), :])

    # gpsimd queue: feat chunks (k/v projections)
    feat_c = [load_chunk(nc.gpsimd, feat_d, N, ncc * 512, f"feat{ncc}")
              for ncc in range(N // 512)]

    # PE warm-up burst: ~3.4us of cold-rate matmuls trips the HAM clock
    # gate to 2.4 GHz before the real compute begins; the remainder keeps
    # the PE busy while the input DMAs land.
    pe_filler(12)

    # Residual features (query half, fp32) - needed only by finalize;
    # loaded mid-kernel on the idle gpsimd queue.
    feat_res = persist.tile([P, 2, NL], F32)

    # Projection outputs; partitions 32..63 etc. hold replicas of 0..31.
    q_rep = persist.tile([P, NL], BF16)          # q[ck, i] x4
    k_rep = persist.tile([P, N], BF16)           # k[ck, j] x4
    vT_sb = persist.tile([P, NJT, C + 1], BF16)  # vT[j%128, jt, c] (+ones col)
    out_acc = persist.tile([P, NIT, C + 1], F32)

    # ---- weight transposes via PE ----
    for ct in range(2):
        ps = tpp.tile([P, 2, P], F32, tag="tp", name="wtp")
        nc.tensor.transpose(ps[:, 0, 0:CK], wq_raw[:, ts(ct, P)],
                            ident[0:CK, 0:CK])
        for rr in range(4):
            nc.vector.tensor_copy(wq_t[:, ct, ts(rr, CK)], ps[:, 0, 0:CK])
        ps = tpp.tile([P, 2, P], F32, tag="tp", name="wtp")
        nc.tensor.transpose(ps[:, 0, 0:CK], wk_raw[:, ts(ct, P)],
                            ident[0:CK, 0:CK])
        for rr in range(4):
            nc.vector.tensor_copy(wk_t[:, ct, ts(rr, CK)], ps[:, 0, 0:CK])
    for cb in range(2):
        for ct in range(2):
            ps = tpp.tile([P, 2, P], F32, tag="tp", name="wtp")
            nc.tensor.transpose(ps[:, 0, :], wv_raw[:, cb, ts(ct, P)], ident[:])
            nc.vector.tensor_copy(wv_t[:, ct, ts(cb, P)], ps[:, 0, :])

    # ---- q/k projections in 512-col chunks through the pv psum pool ----
    def qk_chunk(wt, bias_c, src_chunk, dst, dcol):
        ps = pvp.tile([P, 512], F32, tag="pv", name="qk_ps")
        for ct in range(2):
            nc.tensor.matmul(ps[:], wt[:, ct, :], src_chunk[:, ct, :],
                             start=(ct == 0), stop=(ct == 1))
        nc.vector.tensor_scalar(dst[:, dcol:dcol + 512], ps[:],
                                bias_c[:], None, op0=ADD)

    for icc in range(4):
        qk_chunk(wq_t, bq_c, cond_c[icc], q_rep, icc * 512)
    for ncc in range(4):
        qk_chunk(wk_t, bk_c, feat_c[ncc], k_rep, ncc * 512)

    # ---- vT projection unit ----
    def vt_unit(jt):
        v_ps = pvp.tile([P, C + 1], F32, tag="pv", name="v_ps")
        ch, jl = divmod(jt, 4)
        for ct in range(2):
            nc.tensor.matmul(
                v_ps[:], feat_c[ch][:, ct, ts(jl, P)],
                wv_t[:, ct, :],
                start=(ct == 0), stop=(ct == 1))
        nc.vector.tensor_tensor(vT_sb[:, jt, :], v_ps[:], bv_b[:],
                                op=ADD)

    # ---- phase B units ----
    stage_tiles = {}

    def finalize(it):
        # out[i, c] /= out[i, 256]; transpose to [c, i]; residual + gamma.
        # The normalization multiply runs on the Scalar engine (idle once
        # the exps are done) via activation's per-partition scale operand.
        rcp = finp.tile([P, 1], F32, tag="rcp")
        nc.vector.reciprocal(rcp[:], out_acc[:, it, C:C + 1])
        on = finp.tile([P, C], F32, tag="on")
        nc.scalar.activation(on[:], out_acc[:, it, 0:C],
                             mybir.ActivationFunctionType.Copy,
                             scale=rcp[:])
        qt, sl = divmod(it, 4)
        if sl == 0:
            st_tile = stagep.tile([P, 2, 512], F32, tag="stage")
            stage_tiles[qt] = st_tile
        st = stage_tiles[qt]
        tp = tpp.tile([P, 2, P], F32, tag="tp", name="tp")
        for ct in range(2):
            nc.tensor.transpose(tp[:, ct, :], on[:, ts(ct, P)], ident[:])
        nc.vector.tensor_tensor(
            st[:, :, ts(sl, P)], tp[:, :, :],
            feat_res[:, :, ts(it, P)], op=ADD)
        if sl == 3:
            for ct in range(2):
                nc.scalar.dma_start(out=out_d[ts(ct, P), ts(qt, 512)],
                                    in_=st[:, ct, :])

    def energy_half(attnT, jt, jl, half):
        # half 0 -> queries 0..1023 (PE row groups 0,32; psum A)
        # half 1 -> queries 1024..2047 (row groups 64,96; psum B)
        pool, tag = (epA, "eA") if half == 0 else (epB, "eB")
        e_ps = pool.tile([P, 1024], F32, tag=tag, name=tag)
        for sr in range(2):
            rr = 2 * half + sr
            nc.tensor.matmul(
                e_ps[:, ts(sr, 512)],
                k_rep[ts(rr, CK), ts(jt, P)],
                q_rep[ts(rr, CK), ts(rr, 512)],
                start=True, stop=True, tile_position=(32 * rr, 0))
        nc.scalar.activation(
            attnT[:, jl, ts(half, 1024)], e_ps[:], EXP)

    def pv_unit(attnT, g, it):
        pv = pvp.tile([P, C + 1], F32, tag="pv", name="pv")
        for jl in range(GJ):
            nc.tensor.matmul(
                pv[:], attnT[:, jl, ts(it, P)], vT_sb[:, g * GJ + jl, :],
                start=(jl == 0), stop=(jl == GJ - 1))
        if g == 0:
            nc.vector.tensor_copy(out_acc[:, it, :], pv[:])
        else:
            nc.vector.tensor_tensor(out_acc[:, it, :], pv[:],
                                    out_acc[:, it, :],
                                    op=ADD)
            if g == NG - 1:
                finalize(it)

    # Software pipeline: group g's energy/exp halves interleave with group
    # g-1's PV units (one half-unit + one pv unit per step) so the PE
    # always has dense matmul work. The prologue group (g == 0)
    # interleaves the vT projections and the second k half instead.
    attnTs = {}
    for g in range(NG + 1):
        if g == 2:
            # residual features, needed from the g=3 finalizes onward
            for ct in range(2):
                nc.gpsimd.dma_start(out=feat_res[:, ct, :],
                                    in_=fres_d[ts(ct, P), :])
        if g < NG:
            attnT_t = attnp.tile([P, GJ, NL], BF16, name="attnT")
            attnTs[g] = attnT_t
        for step in range(16):
            if g < NG:
                jl, half = divmod(step, 2)
                energy_half(attnTs[g], g * GJ + jl, jl, half)
            if g == 0:
                vt_unit(2 * step)
                vt_unit(2 * step + 1)
                if step in (8, 10, 12, 14):
                    ncc = 4 + (step - 8) // 2
                    qk_chunk(wk_t, bk_c, feat_c[ncc], k_rep, ncc * 512)
                else:
                    # keep the PE busy enough that the HAM clock gate
                    # does not re-throttle during the ACT-bound g0 phase
                    pe_filler(2)
            else:
                pv_unit(attnTs[g - 1], g - 1, step)


def _split_ctrl_waits(nc, cap=1):
    """Walrus in this image allows only ONE sync-wait command per
    instruction; Tile emits several on phase-boundary instructions (and one
    per live semaphore on the kernel-tail drain). Splitting the excess waits
    onto preceding same-engine NoOps is semantically identical (engine
    sequencers execute in order, so waiting on A then B == waiting on both)."""
    for fn in nc.m.functions:
        for bb in fn.blocks:
            insts = bb.instructions
            out = []
            changed = False
            for ins in insts:
                si = ins.sync_info
                if si is not None and si.on_wait and len(si.on_wait) > cap:
                    waits = list(si.on_wait)
                    for i, w in enumerate(waits[:-cap]):
                        nop = mybir.InstNoOp(
                            name=f"{ins.name}-w{i}",
                            engine=ins.engine,
                            ins=[], outs=[],
                            sync_info=mybir.SyncInfo(on_wait=[w], on_update=[]),
                        )
                        if hasattr(nc, "register_instruction"):
                            nc.register_instruction(nop, overwrite=True)
                        out.append(nop)
                    ins.sync_info = mybir.SyncInfo(
                        on_wait=waits[-cap:], on_update=list(si.on_update))
                    changed = True
                out.append(ins)
            if changed:
                insts[:] = out


def build_nc():
    nc = bass.Bass()
    with tile.TileContext(nc) as tc, ExitStack() as ctx:
        _emit(tc, ctx)
    _split_ctrl_waits(nc)
    return nc


def make_in_maps(features, conditions, Wq, bq, Wk, bk, Wv, bv, gamma):
    import ml_dtypes
    feat = np.ascontiguousarray(np.asarray(features, np.float32).reshape(B, C, N))
    cond = np.ascontiguousarray(np.asarray(conditions, np.float32).reshape(B, C, N))
    feat_bf = feat.astype(ml_dtypes.bfloat16)
    cond_bf = cond.astype(ml_dtypes.bfloat16)
    g = np.float32(np.asarray(gamma, np.float32).reshape(()))
    wq = np.ascontiguousarray(np.asarray(Wq, np.float32))
    wk = np.ascontiguousarray(np.asarray(Wk, np.float32))
    # gamma folded into the v projection (see _emit)
    wv = np.ascontiguousarray(np.asarray(Wv, np.float32) * g)
    bq_ = np.ascontiguousarray(np.asarray(bq, np.float32))
    bk_ = np.ascontiguousarray(np.asarray(bk, np.float32))
    bv_ = np.ascontiguousarray(np.asarray(bv, np.float32) * g)
    in_maps = []
    for core in range(NCORES):
        b, h = divmod(core, 2)
        n0 = h * NL
        in_maps.append({
            "feat": feat_bf[b],
            "cond": np.ascontiguousarray(cond_bf[b][:, n0:n0 + NL]),
            "fres": np.ascontiguousarray(feat[b][:, n0:n0 + NL]),
            "Wq": wq, "Wk": wk, "Wv": wv,
            "bq": bq_, "bk": bk_, "bv": bv_,
        })
    return in_maps


def kernel(features, conditions, Wq, bq, Wk, bk, Wv, bv, gamma):
    global LAST_EXEC_TIME_NS, LAST_TRACE
    in_maps = make_in_maps(features, conditions, Wq, bq, Wk, bk, Wv, bv, gamma)
    nc = build_nc()
    trace = os.environ.get("BASS_KERNEL_TRACE", "0") == "1"
    res = run_bass_kernel_spmd(nc, in_maps, list(range(NCORES)), trace=trace)
    LAST_EXEC_TIME_NS = res.exec_time_ns
    LAST_TRACE = res.instructions_and_trace
    out = np.empty((B, C, N), np.float32)
    for core in range(NCORES):
        b, h = divmod(core, 2)
        out[b][:, h * NL:(h + 1) * NL] = res.results[core]["out"]
    return out.reshape(B, C, H, W)
